# revision 26
# baseline (speedup 1.0000x reference)
"""Cascading sparse attention (GQA decode) on 8 Trainium2 NeuronCores.

Sharding: tensor-parallel over heads. Core c owns q-heads 4c..4c+3 and
kv-head c (Wq/Wk/Wv column slices, Wo row slice, k/v_cache head slice).
Each core computes a partial output (16, 4096); host sums the 8 partials.

The cascading gather at a fixed position decomposes into strided row
ranges of the cache (sink contig / far stride-4 / mid stride-2 / recent
contig), so the device gather is 4 strided DMAs per (batch, cache) — no
indirect DMA. Slot-padding invalidity and the duplicated-row multiplicity
are folded into one additive logit-bias row (ln(weight), -1e30 for pads).
"""

import functools
import math
import sys
from collections import Counter
from contextlib import ExitStack

import numpy as np

sys.path.insert(0, "/opt/trn_rl_repo")

import concourse.bass as bass  # noqa: E402
import concourse.bacc as bacc  # noqa: E402
import concourse.tile as tile  # noqa: E402
from concourse import mybir  # noqa: E402
from concourse import masks  # noqa: E402
from concourse import bass_utils  # noqa: E402

F32 = mybir.dt.float32

SINK, RECENT, MID_W, MID_S, FAR_W, FAR_S = 4, 512, 512, 2, 1536, 4
MAX_CTX = 8192
LN_EPS = 1e-5

B = 16
HID = 4096
H, HKV, D = 32, 8, 128
NCORES = 8
HL = H // NCORES          # 4 local q heads
REP = H // HKV            # 4

# Slot layout: 18 tiles of 128 slots. tile0 = sink(4) + k_new(1) + pad.
# tiles 1-9 far (slot i = 9p + t), tiles 10-13 mid (j = 4p + t),
# tiles 14-17 recent (j = 4p + t).
NT_FAR, NT_MID, NT_REC = 9, 4, 4
NT = 1 + NT_FAR + NT_MID + NT_REC     # 18
SP = NT * 128                          # 2304 padded slots
NEW_SLOT = SINK                        # slot 4 holds k_new/v_new
LOGIT_NJ = 6                           # 6 x 384 logit chunks
LOGIT_W = SP // LOGIT_NJ               # 384


def build_gather_indices(position: int) -> np.ndarray:
    L = position + 1
    idxs = list(range(min(SINK, L))) + [0] * max(0, SINK - L)
    recent_start = max(SINK, L - RECENT)
    r = list(range(recent_start, L))
    while len(r) < RECENT:
        r.insert(0, recent_start)
    idxs += r[-RECENT:]
    mid_end = recent_start
    mid_start = max(SINK, mid_end - MID_W * MID_S)
    m = list(range(mid_start, mid_end, MID_S))
    while len(m) < MID_W:
        m.insert(0, mid_start)
    idxs += m[-MID_W:]
    far_end = mid_start
    far_start = max(SINK, far_end - FAR_W * FAR_S)
    f = list(range(far_start, far_end, FAR_S))
    while len(f) < FAR_W:
        f.insert(0, far_start)
    idxs += f[-FAR_W:]
    return np.asarray(idxs, dtype=np.int64)


def _slot_rows(far_start: int, mid_start: int, recent_start: int,
               n_far: int, n_mid: int, n_rec: int) -> np.ndarray:
    """slot -> cache row (or -1 invalid, -2 new-token slot)."""
    rows = np.full(SP, -1, dtype=np.int64)
    rows[0:SINK] = np.arange(SINK)
    rows[NEW_SLOT] = -2
    for p in range(128):
        for t in range(NT_FAR):
            i = NT_FAR * p + t
            if i < n_far:
                rows[128 * (1 + t) + p] = far_start + FAR_S * i
        for t in range(NT_MID):
            j = NT_MID * p + t
            if j < n_mid:
                rows[128 * (1 + NT_FAR + t) + p] = mid_start + MID_S * j
            if j < n_rec:
                rows[128 * (1 + NT_FAR + NT_MID + t) + p] = recent_start + j
    return rows


def _plan(position: int):
    """Segment offsets + additive logit-bias row for this position."""
    L = position + 1
    recent_start = max(SINK, L - RECENT)
    mid_start = max(SINK, recent_start - MID_W * MID_S)
    far_start = max(SINK, mid_start - FAR_W * FAR_S)
    n_rec = L - recent_start
    n_mid = (recent_start - mid_start + MID_S - 1) // MID_S
    n_far = (mid_start - far_start + FAR_S - 1) // FAR_S
    assert n_rec == RECENT and n_mid == MID_W, "kernel assumes full mid/recent"
    assert 0 < n_far <= NT_FAR * 128
    assert far_start + FAR_S * (NT_FAR * 128 - 1) < MAX_CTX
    rows = _slot_rows(far_start, mid_start, recent_start, n_far, n_mid, n_rec)

    counts = Counter(build_gather_indices(position).tolist())
    mask = np.full(SP, -1e30, dtype=np.float32)
    mask[NEW_SLOT] = 0.0
    got = Counter()
    for s in range(SP):
        r = int(rows[s])
        if r >= 0:
            w = counts[r]
            assert w >= 1, f"slot {s} row {r} not in reference gather"
            mask[s] = math.log(w) if w > 1 else 0.0
            got[r] += 1
    assert set(got) == set(counts), "slot map does not cover reference rows"
    assert all(v == 1 for v in got.values()), "duplicate slots for a row"
    return far_start, mid_start, recent_start, mask


@functools.lru_cache(maxsize=4)
def _build_program(far_start: int, mid_start: int, recent_start: int):
    nc = bacc.Bacc("TRN2", target_bir_lowering=False, debug=False,
                   enable_asserts=False, num_devices=NCORES)

    x_d = nc.dram_tensor("x", (B, HID), F32, kind="ExternalInput").ap()
    kc_d = nc.dram_tensor("kc", (B, MAX_CTX, D), F32, kind="ExternalInput").ap()
    vc_d = nc.dram_tensor("vc", (B, MAX_CTX, D), F32, kind="ExternalInput").ap()
    wq_d = nc.dram_tensor("wq", (HID, HL * D), F32, kind="ExternalInput").ap()
    wkv_d = nc.dram_tensor("wkv", (HID, 2 * D), F32, kind="ExternalInput").ap()
    wo_d = nc.dram_tensor("wo", (HL * D, HID), F32, kind="ExternalInput").ap()
    cs_d = nc.dram_tensor("cs", (B, D // 2), F32, kind="ExternalInput").ap()
    sn_d = nc.dram_tensor("sn", (B, D // 2), F32, kind="ExternalInput").ap()
    qg_d = nc.dram_tensor("qg", (B, HL * D), F32, kind="ExternalInput").ap()
    qb_d = nc.dram_tensor("qb", (B, HL * D), F32, kind="ExternalInput").ap()
    kg_d = nc.dram_tensor("kg", (B, D), F32, kind="ExternalInput").ap()
    kb_d = nc.dram_tensor("kb", (B, D), F32, kind="ExternalInput").ap()
    mask_d = nc.dram_tensor("mask", (HL, SP), F32, kind="ExternalInput").ap()
    out_d = nc.dram_tensor("out", (B, HID), F32, kind="ExternalOutput").ap()

    NQKV = HL * D + 2 * D          # 768 fused q|k|v columns
    QOFF, KOFF, VOFF = 0, HL * D, HL * D + D
    SHIFT = 20.0                   # constant softmax shift (exp(s - SHIFT))
    SCALE = 1.0 / math.sqrt(D)

    with tile.TileContext(nc) as tc, ExitStack() as ctx:
        consts = ctx.enter_context(tc.tile_pool(name="consts", bufs=1))
        persist = ctx.enter_context(tc.tile_pool(name="persist", bufs=1))
        small = ctx.enter_context(tc.tile_pool(name="small", bufs=4))
        wqp = ctx.enter_context(tc.tile_pool(name="wqp", bufs=4))
        wkvp = ctx.enter_context(tc.tile_pool(name="wkvp", bufs=4))
        stg = ctx.enter_context(tc.tile_pool(name="stg", bufs=2))
        ptp = ctx.enter_context(tc.tile_pool(name="ptp", bufs=2))
        ostp = ctx.enter_context(tc.tile_pool(name="ostp", bufs=2))
        ocp = ctx.enter_context(tc.tile_pool(name="ocp", bufs=8))
        wop = ctx.enter_context(tc.tile_pool(name="wop", bufs=3))
        # PSUM budget (8 banks): big-transpose tag 3, q-acc 1, kv-acc 1,
        # logits 2, attn-out 1.
        psB = ctx.enter_context(tc.tile_pool(name="psB", bufs=3, space="PSUM"))
        psQ = ctx.enter_context(tc.tile_pool(name="psQ", bufs=1, space="PSUM"))
        psKV = ctx.enter_context(
            tc.tile_pool(name="psKV", bufs=1, space="PSUM"))
        psL = ctx.enter_context(tc.tile_pool(name="psL", bufs=2, space="PSUM"))
        psO = ctx.enter_context(tc.tile_pool(name="psO", bufs=1, space="PSUM"))

        ident = consts.tile([128, 128], F32, tag="ident")
        masks.make_identity(nc, ident[:])
        eps_sb = consts.tile([B, 1], F32, tag="eps")
        nc.vector.memset(eps_sb, LN_EPS)
        shift_sb = consts.tile([HL, 1], F32, tag="shift")
        nc.vector.memset(shift_sb, -SHIFT)
        cs_sb = consts.tile([B, D // 2], F32, tag="cs")
        sn_sb = consts.tile([B, D // 2], F32, tag="sn")
        nc.gpsimd.dma_start(out=cs_sb, in_=cs_d)
        nc.gpsimd.dma_start(out=sn_sb, in_=sn_d)
        qg_sb = consts.tile([B, HL * D], F32, tag="qg")
        qb_sb = consts.tile([B, HL * D], F32, tag="qb")
        kg_sb = consts.tile([B, D], F32, tag="kg")
        kb_sb = consts.tile([B, D], F32, tag="kb")
        for sb, d in ((qg_sb, qg_d), (qb_sb, qb_d), (kg_sb, kg_d), (kb_sb, kb_d)):
            nc.gpsimd.dma_start(out=sb, in_=d)
        mask_sb = consts.tile([HL, SP], F32, tag="mask")
        nc.gpsimd.dma_start(out=mask_sb, in_=mask_d)

        qkv2 = persist.tile([B, NQKV], F32, tag="qkv2")
        qT = persist.tile([128, HL * B], F32, tag="qT")
        attnT = persist.tile([128, HL * B], F32, tag="attnT")
        kbufs = [persist.tile([128, SP], F32, tag=f"kbuf{i}", name=f"kbuf{i}")
                 for i in range(2)]
        vbufs = [persist.tile([128, SP], F32, tag=f"vbuf{i}", name=f"vbuf{i}")
                 for i in range(2)]
        ktb = persist.tile([128, SP], F32, tag="ktb")
        # sink rows + new-token row staged once: [5, b, d]; row 4 = k/v_new
        knews = persist.tile([SINK + 1, B, D], F32, tag="knews")
        vnews = persist.tile([SINK + 1, B, D], F32, tag="vnews")

        # zero pad slots once: persistent buffers, pads never rewritten.
        # NaN garbage there would poison masked logits / attn accumulation.
        for t_ in kbufs + vbufs:
            nc.vector.memset(t_, 0.0)

        nc.gpsimd.dma_start(out=knews[0:SINK, :, :],
                            in_=kc_d[:, 0:SINK, :].rearrange("b p d -> p b d"))
        nc.gpsimd.dma_start(out=vnews[0:SINK, :, :],
                            in_=vc_d[:, 0:SINK, :].rearrange("b p d -> p b d"))

        # ---- Phase A: QKV projection + LN + RoPE -------------------------
        x_sb = persist.tile([B, HID], F32, tag="x")
        nc.gpsimd.dma_start(out=x_sb, in_=x_d)
        xT = persist.tile([128, 32 * B], F32, tag="xT")
        for c in range(32):
            pst = psB.tile([128, 128], F32, tag="big")
            nc.tensor.transpose(pst[:, :B], x_sb[:, 128 * c:128 * (c + 1)],
                                ident[:B, :B])
            nc.vector.tensor_copy(out=xT[:, B * c:B * (c + 1)],
                                  in_=pst[:, :B])

        ps_q = psQ.tile([B, HL * D], F32, tag="q")
        ps_kv = psKV.tile([B, 2 * D], F32, tag="kv")
        for c in range(32):
            wqc = wqp.tile([128, HL * D], F32, tag="wq")
            wkvc = wkvp.tile([128, 2 * D], F32, tag="wkv")
            nc.gpsimd.dma_start(out=wqc, in_=wq_d[128 * c:128 * (c + 1), :])
            nc.gpsimd.dma_start(out=wkvc, in_=wkv_d[128 * c:128 * (c + 1), :])
            lhsT = xT[:, B * c:B * (c + 1)]
            st, sp = (c == 0), (c == 31)
            nc.tensor.matmul(ps_q, lhsT, wqc, start=st, stop=sp)
            nc.tensor.matmul(ps_kv, lhsT, wkvc, start=st, stop=sp)

        qkv = persist.tile([B, NQKV], F32, tag="qkv")
        nc.vector.tensor_copy(out=qkv[:, QOFF:QOFF + HL * D], in_=ps_q)
        nc.vector.tensor_copy(out=qkv[:, KOFF:KOFF + 2 * D], in_=ps_kv)

        # per-head layernorm over D
        for j in range(HL + 2):
            blk = qkv[:, D * j:D * (j + 1)]
            st6 = small.tile([B, 6], F32, tag="st6")
            mv = small.tile([B, 2], F32, tag="mv")
            nc.vector.bn_stats(out=st6, in_=blk)
            nc.vector.bn_aggr(out=mv, in_=st6)
            nc.scalar.activation(out=mv[:, 1:2], in_=mv[:, 1:2],
                                 func=mybir.ActivationFunctionType.Sqrt,
                                 bias=eps_sb, scale=1.0)
            nc.vector.reciprocal(out=mv[:, 1:2], in_=mv[:, 1:2])
            nc.vector.tensor_scalar(out=blk, in0=blk,
                                    scalar1=mv[:, 0:1], scalar2=mv[:, 1:2],
                                    op0=mybir.AluOpType.subtract,
                                    op1=mybir.AluOpType.mult)
            if j < HL:
                g = qg_sb[:, D * j:D * (j + 1)]
                bta = qb_sb[:, D * j:D * (j + 1)]
            elif j == HL:
                g, bta = kg_sb, kb_sb
            else:
                g = bta = None
            if g is not None:
                nc.vector.tensor_mul(out=blk, in0=blk, in1=g)
                nc.vector.tensor_add(out=blk, in0=blk, in1=bta)

        # RoPE on q heads + k (not v); write into qkv2
        for j in range(HL + 1):
            x1 = qkv[:, D * j:D * j + 64]
            x2 = qkv[:, D * j + 64:D * (j + 1)]
            o1 = qkv2[:, D * j:D * j + 64]
            o2 = qkv2[:, D * j + 64:D * (j + 1)]
            t1 = small.tile([B, 64], F32, tag="t1")
            t2 = small.tile([B, 64], F32, tag="t2")
            nc.vector.tensor_mul(out=t1, in0=x1, in1=cs_sb)
            nc.vector.tensor_mul(out=t2, in0=x2, in1=sn_sb)
            nc.vector.tensor_mul(out=o2, in0=x2, in1=cs_sb)
            nc.vector.tensor_sub(out=o1, in0=t1, in1=t2)
            nc.vector.tensor_mul(out=t2, in0=x1, in1=sn_sb)
            nc.vector.tensor_add(out=o2, in0=o2, in1=t2)
        nc.vector.tensor_copy(out=qkv2[:, VOFF:VOFF + D],
                              in_=qkv[:, VOFF:VOFF + D])
        # fold logit scale into q
        nc.scalar.mul(out=qkv2[:, 0:HL * D], in_=qkv2[:, 0:HL * D], mul=SCALE)

        # append k_new/v_new as row 4 of the staging tiles (size-matched
        # DMA: dest [1,16,128] iterates (b,d), src [16,128] iterates (b,d))
        nc.gpsimd.dma_start(out=knews[SINK:SINK + 1, :, :],
                            in_=qkv2[:, KOFF:KOFF + D])
        nc.gpsimd.dma_start(out=vnews[SINK:SINK + 1, :, :],
                            in_=qkv2[:, VOFF:VOFF + D])

        # qT[d, 16h + b] = q[b, h, d] (scaled)
        for h in range(HL):
            pst = psB.tile([128, 128], F32, tag="big")
            nc.tensor.transpose(pst[:, :B], qkv2[:, D * h:D * (h + 1)],
                                ident[:B, :B])
            nc.vector.tensor_copy(out=qT[:, B * h:B * (h + 1)],
                                  in_=pst[:, :B])

        def load_cache(buf, src, news, b):
            # tile 0: sink rows 0..3 + new row at slot 4, one SBUF DMA
            nc.gpsimd.dma_start(out=buf[0:SINK + 1, 0:D], in_=news[:, b, :])
            nc.gpsimd.dma_start(
                out=buf[:, 128:128 * (1 + NT_FAR)]
                    .rearrange("p (t d) -> p t d", d=D),
                in_=src[b, far_start:far_start + FAR_S * 128 * NT_FAR:FAR_S, :]
                    .rearrange("(p t) d -> p t d", t=NT_FAR))
            o = 128 * (1 + NT_FAR)
            nc.gpsimd.dma_start(
                out=buf[:, o:o + 128 * NT_MID]
                    .rearrange("p (t d) -> p t d", d=D),
                in_=src[b, mid_start:mid_start + MID_S * 128 * NT_MID:MID_S, :]
                    .rearrange("(p t) d -> p t d", t=NT_MID))
            o = 128 * (1 + NT_FAR + NT_MID)
            nc.gpsimd.dma_start(
                out=buf[:, o:o + 128 * NT_REC]
                    .rearrange("p (t d) -> p t d", d=D),
                in_=src[b, recent_start:recent_start + 128 * NT_REC, :]
                    .rearrange("(p t) d -> p t d", t=NT_REC))

        # ---- Per-batch attention ----------------------------------------
        for b in range(B):
            kb_t = kbufs[b % 2]
            vb_t = vbufs[b % 2]
            load_cache(kb_t, kc_d, knews, b)
            load_cache(vb_t, vc_d, vnews, b)
            for t in range(NT):
                pst = psB.tile([128, 128], F32, tag="big")
                nc.tensor.transpose(pst, kb_t[:, 128 * t:128 * (t + 1)],
                                    ident)
                nc.vector.tensor_copy(
                    out=ktb[:, 128 * t:128 * (t + 1)], in_=pst)
            stage = stg.tile([HL, SP], F32, tag="lst", bufs=1)
            for j in range(LOGIT_NJ):
                psl = psL.tile([HL, LOGIT_W], F32, tag="l")
                nc.tensor.matmul(psl, qT[:, b:HL * B:B],
                                 ktb[:, LOGIT_W * j:LOGIT_W * (j + 1)],
                                 start=True, stop=True)
                nc.vector.tensor_add(
                    out=stage[:, LOGIT_W * j:LOGIT_W * (j + 1)],
                    in0=psl,
                    in1=mask_sb[:, LOGIT_W * j:LOGIT_W * (j + 1)])
            # unnormalized softmax: exp(s - SHIFT), row-sums via accum_out
            sums = small.tile([HL, 1], F32, tag="sums")
            estage = stg.tile([HL, SP], F32, tag="est")
            nc.scalar.activation(out=estage, in_=stage,
                                 func=mybir.ActivationFunctionType.Exp,
                                 bias=shift_sb, scale=1.0, accum_out=sums)
            rec = small.tile([HL, 1], F32, tag="rec")
            nc.vector.reciprocal(out=rec, in_=sums)
            # P^T per tile, then attention accumulate
            pt_b = ptp.tile([128, HL * NT], F32, tag="pt")
            for t in range(NT):
                psp = psB.tile([128, 128], F32, tag="big")
                nc.tensor.transpose(psp[:, :HL],
                                    estage[:, 128 * t:128 * (t + 1)],
                                    ident[:HL, :HL])
                nc.vector.tensor_copy(out=pt_b[:, HL * t:HL * (t + 1)],
                                      in_=psp[:, :HL])
            pso = psO.tile([HL, D], F32, tag="o")
            for t in range(NT):
                nc.tensor.matmul(pso, pt_b[:, HL * t:HL * (t + 1)],
                                 vb_t[:, 128 * t:128 * (t + 1)],
                                 start=(t == 0), stop=(t == NT - 1))
            ost = ostp.tile([HL, D], F32, tag="ost")
            nc.vector.tensor_scalar_mul(out=ost, in0=pso, scalar1=rec)
            psa = psB.tile([128, 128], F32, tag="big")
            nc.tensor.transpose(psa[:, :HL], ost, ident[:HL, :HL])
            nc.vector.tensor_copy(out=attnT[:, HL * b:HL * (b + 1)],
                                  in_=psa[:, :HL])

        # ---- Output projection ------------------------------------------
        for n in range(8):
            woc = wop.tile([128, HL, 512], F32, tag="woc")
            nc.gpsimd.dma_start(
                out=woc,
                in_=wo_d[:, 512 * n:512 * (n + 1)]
                    .rearrange("(k p) j -> p k j", p=128))
            psw = psQ.tile([B, 512], F32, tag="q")
            for k in range(HL):
                nc.tensor.matmul(
                    psw, attnT[:, k:HL * B:HL], woc[:, k, :],
                    start=(k == 0), stop=(k == HL - 1))
            oc = ocp.tile([B, 512], F32, tag="oc")
            nc.vector.tensor_copy(out=oc, in_=psw)
            nc.sync.dma_start(out=out_d[:, 512 * n:512 * (n + 1)], in_=oc)

    nc.compile()
    return nc


def kernel(**inputs):
    hidden = np.asarray(inputs["hidden_states"], dtype=np.float32)
    k_cache = np.asarray(inputs["k_cache"], dtype=np.float32)
    v_cache = np.asarray(inputs["v_cache"], dtype=np.float32)
    position = int(np.asarray(inputs["position"]))
    rope_cos = np.asarray(inputs["rope_cos"], dtype=np.float32)
    rope_sin = np.asarray(inputs["rope_sin"], dtype=np.float32)
    Wq = np.asarray(inputs["Wq"], dtype=np.float32)
    Wk = np.asarray(inputs["Wk"], dtype=np.float32)
    Wv = np.asarray(inputs["Wv"], dtype=np.float32)
    Wo = np.asarray(inputs["Wo"], dtype=np.float32)
    q_gamma = np.asarray(inputs["q_gamma"], dtype=np.float32)
    q_beta = np.asarray(inputs["q_beta"], dtype=np.float32)
    k_gamma = np.asarray(inputs["k_gamma"], dtype=np.float32)
    k_beta = np.asarray(inputs["k_beta"], dtype=np.float32)

    far_start, mid_start, recent_start, mask_row = _plan(position)
    mask_full = np.ascontiguousarray(
        np.broadcast_to(mask_row, (HL, SP)), dtype=np.float32)
    cs = np.ascontiguousarray(
        np.broadcast_to(rope_cos[position], (B, D // 2)), dtype=np.float32)
    sn = np.ascontiguousarray(
        np.broadcast_to(rope_sin[position], (B, D // 2)), dtype=np.float32)
    qg = np.ascontiguousarray(np.tile(q_gamma, (B, HL)), dtype=np.float32)
    qb = np.ascontiguousarray(np.tile(q_beta, (B, HL)), dtype=np.float32)
    kg = np.ascontiguousarray(np.tile(k_gamma, (B, 1)), dtype=np.float32)
    kbt = np.ascontiguousarray(np.tile(k_beta, (B, 1)), dtype=np.float32)
    x = np.ascontiguousarray(hidden.reshape(B, HID))

    in_maps = []
    for c in range(NCORES):
        in_maps.append({
            "x": x,
            "kc": np.ascontiguousarray(k_cache[:, c]),
            "vc": np.ascontiguousarray(v_cache[:, c]),
            "wq": np.ascontiguousarray(Wq[:, c * HL * D:(c + 1) * HL * D]),
            "wkv": np.ascontiguousarray(np.concatenate(
                [Wk[:, c * D:(c + 1) * D], Wv[:, c * D:(c + 1) * D]], axis=1)),
            "wo": np.ascontiguousarray(Wo[c * HL * D:(c + 1) * HL * D, :]),
            "cs": cs, "sn": sn,
            "qg": qg, "qb": qb, "kg": kg, "kb": kbt,
            "mask": mask_full,
        })

    nc = _build_program(far_start, mid_start, recent_start)
    global _LAST_IN_MAPS
    _LAST_IN_MAPS = in_maps
    res = bass_utils.run_bass_kernel_spmd(
        nc, in_maps, core_ids=list(range(NCORES)))
    global LAST_RESULT
    LAST_RESULT = res
    out = np.zeros((B, HID), dtype=np.float32)
    for r in res.results:
        out += r["out"]
    return out.reshape(B, 1, HID)


LAST_RESULT = None


def timeline_ns(position: int = 6000, trace_path: str | None = None) -> float:
    """Cost-model timeline estimate for one core (no hardware)."""
    from concourse.timeline_sim import TimelineSim

    fs, ms, rs, _ = _plan(position)
    nc = _build_program(fs, ms, rs)
    ts = TimelineSim(nc, trace=trace_path is not None)
    t = ts.simulate()
    if trace_path is not None and ts.perfetto is not None:
        ts.perfetto.save(trace_path)
    return t


def bench_hw(inputs, iters: int = 10):
    """Measure on-device execution wall time with device-resident inputs.

    Mirrors bass2jax.run_bass_via_pjrt's multi-core path, but without
    donation so the jitted function can be re-executed, and with inputs
    device_put once up front. Returns (per_call_s, pipelined_s).
    """
    import time

    import jax
    import jax.numpy as jnp  # noqa: F401
    from jax.sharding import Mesh, NamedSharding, PartitionSpec
    from jax.experimental.shard_map import shard_map

    import concourse.bass2jax as b2j
    from concourse import mybir as mb

    # reuse kernel()'s host prep
    out = kernel(**inputs)  # noqa: F841  (warms program + provides in_maps)
    fs, ms, rs, mask_row = _plan(int(np.asarray(inputs["position"])))
    nc = _build_program(fs, ms, rs)

    partition_name = (nc.partition_id_tensor.name
                      if nc.partition_id_tensor else None)
    in_names, out_names, out_avals, zero_outs = [], [], [], []
    for alloc in nc.m.functions[0].allocations:
        if not isinstance(alloc, mb.MemoryLocationSet):
            continue
        name = alloc.memorylocations[0].name
        if alloc.kind == "ExternalInput":
            if name != partition_name:
                in_names.append(name)
        elif alloc.kind == "ExternalOutput":
            out_names.append(name)
            shape = tuple(alloc.tensor_shape)
            dtype = mb.dt.np(alloc.dtype)
            out_avals.append(jax.core.ShapedArray(shape, dtype))
            zero_outs.append(np.zeros(shape, dtype))
    n_params = len(in_names)
    all_names = in_names + out_names
    if partition_name is not None:
        all_names = all_names + [partition_name]

    def _body(*args):
        operands = list(args)
        if partition_name is not None:
            operands.append(b2j.partition_id_tensor())
        outs = b2j._bass_exec_p.bind(
            *operands,
            out_avals=tuple(out_avals),
            in_names=tuple(all_names),
            out_names=tuple(out_names),
            lowering_input_output_aliases=(),
            sim_require_finite=True,
            sim_require_nnan=True,
            nc=nc,
        )
        return tuple(outs)

    b2j.install_neuronx_cc_hook()
    devices = jax.devices()[:NCORES]
    mesh = Mesh(np.asarray(devices), ("core",))
    spec = PartitionSpec("core")
    n_out = len(out_names)
    fn = jax.jit(
        shard_map(_body, mesh=mesh, in_specs=(spec,) * (n_params + n_out),
                  out_specs=(spec,) * n_out, check_rep=False),
        keep_unused=True,
    )

    in_maps = _LAST_IN_MAPS
    concat_in = [
        np.concatenate([np.asarray(in_maps[c][nm]) for c in range(NCORES)], 0)
        for nm in in_names
    ]
    concat_zero = [
        np.zeros((NCORES * z.shape[0], *z.shape[1:]), z.dtype)
        for z in zero_outs
    ]
    sharding = NamedSharding(mesh, spec)
    dev_in = [jax.device_put(a, sharding) for a in concat_in]
    dev_zero = [jax.device_put(a, sharding) for a in concat_zero]
    jax.block_until_ready(dev_in)

    r = fn(*dev_in, *dev_zero)
    jax.block_until_ready(r)
    r = fn(*dev_in, *dev_zero)
    jax.block_until_ready(r)

    t0 = time.perf_counter()
    for _ in range(iters):
        r = fn(*dev_in, *dev_zero)
        jax.block_until_ready(r)
    per_call = (time.perf_counter() - t0) / iters

    t0 = time.perf_counter()
    rs_ = [fn(*dev_in, *dev_zero) for _ in range(iters)]
    jax.block_until_ready(rs_)
    pipelined = (time.perf_counter() - t0) / iters
    return per_call, pipelined


_LAST_IN_MAPS = None
